# revision 52
# baseline (speedup 1.0000x reference)
"""BinaryDense kernel for Trainium2: out = sign(x) @ sign(w).

x: [8192, 2048] f32, w: [2048, 2048] f32 -> out: [8192, 2048] f32.

Strategy: data-parallel shard of the batch dim across 8 NeuronCores
(1024 rows each, w replicated). The host binarizes both inputs to
exact +-1.0 in fp8e4m3 (binarize is a pure elementwise sign; +-1 is
exactly representable in fp8, so this is bit-exact with the reference)
and lays them out so every DMA chunk is a large contiguous block:
  - x shard pre-transposed to x^T, chunked by k-pair (256KB chunks,
    2KB per SBUF partition).
  - w relaid out column-quarter-major and k-quad-chunked; quarter 0
    streams as 256KB quads (the first split into two 128KB pairs),
    quarters 1-3 as 1MB octs via a second AP view of the same bytes
    (fewer tiles -> fewer chain-boundary semaphore waits).

Per core (timeline-driven schedule):
  - w chunks stream on the sync HWDGE ring, x chunks on the scalar
    HWDGE ring (two independent hardware DMA rings); the first chunk
    of each is halved so the first matmul can issue as early as
    possible. Chunks are k-major with each chunk contiguous per SBUF
    partition (1-2KB DMA descriptors, ~214GB/s per ring).
  - A few dummy matmuls on memset data fill the PE while the first
    chunks are in flight (starts the HAM clock-gate busy window).
  - 256 fp8 DoubleRow matmuls (K=256, N=512 each, 216ns issue rate
    warm at 2.4GHz) in 32 PSUM chains: phase 1 runs the 8 chains
    (all m-tiles x output quarter 0) k-outer, paced by chunk
    arrival; phase 2 runs the remaining 24 chains k-inner, quarter-
    major, so each w quarter is consumed long after it lands and
    each chain closes quickly, recycling its PSUM bank.
  - Each closed chain is evicted on the (otherwise idle) Vector
    engine, psum fp32 -> fp16 (sums are integers in [-2048, 2048],
    exactly representable in fp16), and its 128KB output chunk is
    stored immediately, alternating between the two DMA rings ->
    stores overlap compute; the final chain runs as two independent
    half-width chains so the first half's evict+store overlap the
    second half's matmuls, leaving a ~1.1us post-last-matmul tail.

All arithmetic is exact: products are +-1, psum sums are integers
bounded by 2048 (fp32- and fp16-exact). The host fp16->f32 widening
is exact, so the result matches the reference bit-for-bit.
"""

import sys

if "/opt/trn_rl_repo" not in sys.path:
    sys.path.insert(0, "/opt/trn_rl_repo")

import numpy as np

B_FULL, D_IN, UNITS = 8192, 2048, 2048
N_CORES = 8
B_CORE = B_FULL // N_CORES  # 1024
P = 128


def build_kernel(B=B_CORE, D=D_IN, U=UNITS, prewarm=6):
    """Build (and compile) the per-core Bass kernel. Returns the Bacc nc."""
    from concourse import bacc
    import concourse.mybir as mybir
    import concourse.tile as tile

    f32 = mybir.dt.float32
    f8 = mybir.dt.float8e4
    f16 = mybir.dt.float16

    KT = D // P            # k-subtiles (16)
    NT = KT // 2           # k-pairs per chain (8)
    MT = B // P            # m-tiles (8)
    NQ = U // 512          # output column quarters (4)

    nc = bacc.Bacc("TRN2", target_bir_lowering=False)
    # Both inputs arrive chunk-major: each chunk is a [128, 2, cols]
    # tile laid out contiguously per partition (2*cols bytes), so DMA
    # descriptors are 1-2KB instead of 512B.
    x_d = nc.dram_tensor("xT", [NT * P * 2, B], f8, kind="ExternalInput")
    w_d = nc.dram_tensor("w", [NQ * NT * P * 2, 512], f8,
                         kind="ExternalInput")
    o_d = nc.dram_tensor("out", [B, U], f16, kind="ExternalOutput")

    x_ap = x_d[:].rearrange("(c p k) m -> c p k m", p=P, k=2)  # [8,128,2,B]
    w_ap = w_d[:].rearrange("(c p k) u -> c p k u", p=P, k=4)  # [16,128,4,512]
    # same DRAM bytes viewed as 1MB oct chunks (two k-quads): used for
    # quarters 1-3, halving tile count and chain-boundary sem waits
    w_oap = w_d[:].rearrange("(q h t p k) u -> (q h) p t k u",
                             q=NQ, h=2, t=2, p=P, k=4)       # [8,128,2,4,512]
    o_ap = o_d[:].rearrange("(j p) u -> j p u", p=P)           # [MT, 128, U]

    DR = mybir.MatmulPerfMode.DoubleRow

    with tile.TileContext(nc) as tc, \
         tc.tile_pool(name="resident", bufs=1) as resident, \
         tc.tile_pool(name="mpsum", bufs=8, space="PSUM") as mpsum:

        xk = [resident.tile([P, 2, B], f8, name=f"xk_{t}")
              for t in range(1, NT)]
        xk.insert(0, None)
        # first x chunk split in half so the first matmul can start
        # ~1us earlier
        xk0h = [resident.tile([P, 2, B // 2], f8, name=f"xk0_{h}")
                for h in range(2)]
        # w quarter 0 arrives as k-quads (2KB/partition descriptors),
        # the first quad split into two pair tiles so the first
        # matmul can start early; quarters 1-3 arrive as 1MB octs
        # (same 2KB descriptors, fewer tiles).
        wq4 = [None if t == 0 else
               resident.tile([P, 4, 512], f8, name=f"wq4_0_{t}")
               for t in range(NT // 2)]
        wq0h = [resident.tile([P, 2, 512], f8, name=f"wq0_{h}")
                for h in range(2)]
        wq8o = [[resident.tile([P, 2, 4, 512], f8, name=f"wq8o_{q}_{h}")
                 for h in range(2)] for q in range(1, NQ)]
        ost = [resident.tile([P, U], f16, name=f"ost_{j}")
               for j in range(MT)]

        # ---- input DMAs: w on the sync ring, x on the scalar ring.
        # (Interleaving early x chunks onto the sync ring was tried
        # and regressed: the first ~5us of DMA bandwidth is a shared
        # ramp, so it only starved the scalar ring's x chunks.) ----
        # (A 1KB priming transfer at the head of each ring was tried
        # to absorb the ~1.5us issue-to-first-packet latency; the
        # latency is per-transfer descriptor pipelining, not ring
        # wake-up, so priming only delayed the real chunks.)
        nc.sync.dma_start(wq0h[0], w_ap[0][:, 0:2, :])
        nc.scalar.dma_start(xk0h[0], x_ap[0][:, :, :B // 2])
        nc.scalar.dma_start(xk0h[1], x_ap[0][:, :, B // 2:])
        nc.sync.dma_start(wq0h[1], w_ap[0][:, 2:4, :])
        for t in range(NT // 2):
            if t > 0:
                nc.sync.dma_start(wq4[t], w_ap[t])
                nc.scalar.dma_start(xk[2 * t], x_ap[2 * t])
            nc.scalar.dma_start(xk[2 * t + 1], x_ap[2 * t + 1])
        for q in range(1, NQ):
            for h in range(2):
                nc.sync.dma_start(wq8o[q - 1][h], w_oap[q * 2 + h])

        # ---- PE prewarm: dummy matmuls on memset data fill the PE
        # while the first input chunks are in flight and start the HAM
        # clock-gate busy window early. Sized to end right as the
        # first chunks land (dummies queue ahead of real matmuls, so
        # more is not better). ----
        if prewarm:
            # Memset-gated dummies start at ~8.5us, leaving the PE
            # cold (1.2GHz) until ~13.8us -- and that is intentional:
            # a PE warmed earlier (tried via memset-free dummies)
            # consumes chunks at 222GB/s, starves on the DMA ramp
            # (~50-150GB/s until ~14us), and the resulting stall
            # re-throttles the clock gate for another ~7us window.
            # The cold-PE chunk demand (~112GB/s) matches the ramp.
            dl = resident.tile([P, 2, P], f8, name="warm_l")
            dr_ = resident.tile([P, 2, 512], f8, name="warm_r")
            nc.gpsimd.memset(dl, 0)
            nc.gpsimd.memset(dr_, 0)
            wps = mpsum.tile([P, 512], f32, tag="ps", name="warm_ps")
            for i in range(prewarm):
                nc.tensor.matmul(wps, lhsT=dl, rhs=dr_,
                                 start=True, stop=True, perf_mode=DR)

        def mm(ps, j, q, t):
            if t == 0:
                lhsT = xk0h[j // 4][:, :, P * (j % 4):P * (j % 4 + 1)]
            else:
                lhsT = xk[t][:, :, P * j:P * (j + 1)]
            if q == 0:
                if t < 2:
                    rhs = wq0h[t]
                else:
                    kc = (t % 2) * 2
                    rhs = wq4[t // 2][:, kc:kc + 2, :]
            else:
                ti = t % 4
                kc = (ti % 2) * 2
                rhs = wq8o[q - 1][t // 4][:, ti // 2, kc:kc + 2, :]
            nc.tensor.matmul(
                ps, lhsT=lhsT, rhs=rhs,
                start=(t == 0), stop=(t == NT - 1),
                perf_mode=DR,
            )

        def evict_store(ps, j, q, store_engine=None, split=False):
            dst = ost[j][:, 512 * q:512 * (q + 1)]
            if split:
                # final chain: halve latency by evicting on both the
                # Vector and Scalar engines and storing the halves on
                # both DMA rings in parallel
                h = 256
                nc.vector.tensor_copy(dst[:, :h], ps[:, :h])
                nc.scalar.activation(
                    dst[:, h:], ps[:, h:],
                    mybir.ActivationFunctionType.Copy)
                nc.sync.dma_start(
                    o_ap[j, :, 512 * q:512 * q + h], dst[:, :h])
                nc.scalar.dma_start(
                    o_ap[j, :, 512 * q + h:512 * (q + 1)], dst[:, h:])
            else:
                nc.vector.tensor_copy(dst, ps)
                (store_engine or nc.scalar).dma_start(
                    o_ap[j, :, 512 * q:512 * (q + 1)], dst)

        # ---- phase 1: chains (j, q0), k-pair-outer, paced by arrival ----
        ps0 = [mpsum.tile([P, 512], f32, tag="ps", name=f"ps_{j}_0")
               for j in range(MT)]
        for t in range(NT):
            for j in range(MT):
                mm(ps0[j], j, 0, t)
        for j in range(MT):
            evict_store(ps0[j], j, 0)

        # ---- phase 2: chains (j, q1..3), q-major so each w quarter
        # is consumed long after it lands; stores alternate between
        # the two DMA rings (the sync ring is idle once w has landed)
        for q in range(1, NQ):
            for j in range(MT):
                if q == NQ - 1 and j == MT - 1:
                    continue
                ps = mpsum.tile([P, 512], f32, tag="ps",
                                name=f"ps_{j}_{q}")
                for t in range(NT):
                    mm(ps, j, q, t)
                evict_store(ps, j, q,
                            store_engine=nc.sync if j % 2 else nc.scalar)

        # The final chain runs as two independent half-width (N=256)
        # chains: the first half's evict + store overlap the second
        # half's matmuls, so only a 256-col evict and a 64KB store
        # remain after the last matmul.
        j, q = MT - 1, NQ - 1
        for h in range(2):
            ph = mpsum.tile([P, 512], f32, tag="ps", name=f"ps_f{h}")
            for t in range(NT):
                ti = t % 4
                kc = (ti % 2) * 2
                nc.tensor.matmul(
                    ph[:, :256],
                    lhsT=(xk0h[1][:, :, 3 * P:4 * P] if t == 0
                          else xk[t][:, :, P * j:P * (j + 1)]),
                    rhs=wq8o[q - 1][t // 4][:, ti // 2, kc:kc + 2,
                                            256 * h:256 * (h + 1)],
                    start=(t == 0), stop=(t == NT - 1),
                    perf_mode=DR,
                )
            dst = ost[j][:, 512 * q + 256 * h:512 * q + 256 * (h + 1)]
            if h == 0:
                nc.scalar.activation(dst, ph[:, :256],
                                     mybir.ActivationFunctionType.Copy)
                nc.sync.dma_start(
                    o_ap[j, :, 512 * q:512 * q + 256], dst)
            else:
                nc.vector.tensor_copy(dst, ph[:, :256])
                nc.scalar.dma_start(
                    o_ap[j, :, 512 * q + 256:512 * (q + 1)], dst)

    nc.compile()
    return nc


_NC_CACHE = {}
LAST_RESULTS = {}


def _get_nc(**kwargs):
    key = tuple(sorted(kwargs.items()))
    if key not in _NC_CACHE:
        _NC_CACHE[key] = build_kernel(**kwargs)
    return _NC_CACHE[key]


def kernel(x, w, _trace=False, _trace_cores=None, **build_kwargs):
    from concourse.bass_utils import run_bass_kernel_spmd
    import ml_dtypes

    x = np.asarray(x, dtype=np.float32)
    w = np.asarray(w, dtype=np.float32)
    assert x.shape == (B_FULL, D_IN) and w.shape == (D_IN, UNITS)

    nc = _get_nc(**build_kwargs)

    f8 = ml_dtypes.float8_e4m3
    # Exact host-side binarize: sign(v) with sign(0) -> +1, as +-1.0
    # which fp8e4m3 represents exactly.
    xb = np.where(x >= 0, np.float32(1), np.float32(-1))
    wb = np.where(w >= 0, np.float32(1), np.float32(-1))
    xbT8 = np.ascontiguousarray(xb.T).astype(f8)          # [D, B_FULL]
    # w chunk-major [q, t2, p, s4, u']: chunk (q,t2) holds k-rows
    # 512*t2 + 128*s4 + p of output-column quarter q, contiguous per
    # SBUF partition p (2KB descriptors).
    wq8 = np.ascontiguousarray(
        wb.astype(f8).reshape(4, 4, 128, 4, 512).transpose(3, 0, 2, 1, 4)
    ).reshape(4 * D_IN, 512)

    def x_chunked(xt):
        # [t, p, ko, m]: chunk t holds k-rows 256t + 128*ko + p
        return np.ascontiguousarray(
            xt.reshape(8, 2, 128, B_CORE).transpose(0, 2, 1, 3)
        ).reshape(D_IN, B_CORE)

    in_maps = [
        {"xT": x_chunked(xbT8[:, c * B_CORE:(c + 1) * B_CORE]),
         "w": wq8}
        for c in range(N_CORES)
    ]
    br = run_bass_kernel_spmd(
        nc, in_maps, list(range(N_CORES)),
        trace=_trace, trace_cores=_trace_cores,
    )
    LAST_RESULTS["br"] = br
    out = np.concatenate(
        [br.results[c]["out"].astype(np.float32) for c in range(N_CORES)],
        axis=0,
    )
    return out


if __name__ == "__main__":
    rng = np.random.default_rng(0)
    x = rng.standard_normal((B_FULL, D_IN), dtype=np.float32)
    w = (rng.standard_normal((D_IN, UNITS), dtype=np.float32) * 0.1).astype(
        np.float32
    )
    out = kernel(x, w)
    exp = np.sign(x + (x == 0)) @ np.sign(w + (w == 0))
    print("max abs err:", np.max(np.abs(out - exp)))


# revision 53
# speedup vs baseline: 1.1734x; 1.1734x over previous
"""BinaryDense kernel for Trainium2: out = sign(x) @ sign(w).

x: [8192, 2048] f32, w: [2048, 2048] f32 -> out: [8192, 2048] f32.

Strategy: data-parallel shard of the batch dim across 8 NeuronCores
(1024 rows each, w replicated). The host binarizes both inputs to
exact +-1.0 in fp8e4m3 (binarize is a pure elementwise sign; +-1 is
exactly representable in fp8, so this is bit-exact with the reference)
and lays them out so every DMA chunk is a large contiguous block:
  - x shard pre-transposed to x^T, chunked by k-pair (256KB chunks,
    2KB per SBUF partition).
  - w relaid out column-quarter-major and k-quad-chunked; quarter 0
    streams as 256KB quads (the first split into two 128KB pairs),
    quarters 1-3 as 1MB octs via a second AP view of the same bytes
    (fewer tiles -> fewer chain-boundary semaphore waits).

Per core (timeline-driven schedule):
  - w chunks stream on the sync HWDGE ring, x chunks on the scalar
    HWDGE ring (two independent hardware DMA rings); the first chunk
    of each is halved so the first matmul can issue as early as
    possible. Chunks are k-major with each chunk contiguous per SBUF
    partition (1-2KB DMA descriptors, ~214GB/s per ring).
  - A few dummy matmuls on memset data fill the PE while the first
    chunks are in flight (starts the HAM clock-gate busy window).
  - 256 fp8 DoubleRow matmuls (K=256, N=512 each, 216ns issue rate
    warm at 2.4GHz) in 32 PSUM chains: phase 1 runs the 8 chains
    (all m-tiles x output quarter 0) k-outer, paced by chunk
    arrival; phase 2 runs the remaining 24 chains k-inner, quarter-
    major, so each w quarter is consumed long after it lands and
    each chain closes quickly, recycling its PSUM bank.
  - Each closed chain is evicted on the (otherwise idle) Vector
    engine, psum fp32 -> fp16 (sums are integers in [-2048, 2048],
    exactly representable in fp16), and its 128KB output chunk is
    stored immediately, alternating between the two DMA rings ->
    stores overlap compute; the final chain runs as two independent
    half-width chains so the first half's evict+store overlap the
    second half's matmuls, leaving a ~1.1us post-last-matmul tail.

All arithmetic is exact: products are +-1, psum sums are integers
bounded by 2048 (fp32- and fp16-exact). The host fp16->f32 widening
is exact, so the result matches the reference bit-for-bit.
"""

import sys

if "/opt/trn_rl_repo" not in sys.path:
    sys.path.insert(0, "/opt/trn_rl_repo")

import numpy as np

B_FULL, D_IN, UNITS = 8192, 2048, 2048
N_CORES = 8
B_CORE = B_FULL // N_CORES  # 1024
P = 128


def build_kernel(B=B_CORE, D=D_IN, U=UNITS, prewarm=5):
    """Build (and compile) the per-core Bass kernel. Returns the Bacc nc."""
    from concourse import bacc
    import concourse.mybir as mybir
    import concourse.tile as tile

    f32 = mybir.dt.float32
    f8 = mybir.dt.float8e4
    f16 = mybir.dt.float16

    KT = D // P            # k-subtiles (16)
    NT = KT // 2           # k-pairs per chain (8)
    MT = B // P            # m-tiles (8)
    NQ = U // 512          # output column quarters (4)

    nc = bacc.Bacc("TRN2", target_bir_lowering=False)
    # Both inputs arrive chunk-major: each chunk is a [128, 2, cols]
    # tile laid out contiguously per partition (2*cols bytes), so DMA
    # descriptors are 1-2KB instead of 512B.
    x_d = nc.dram_tensor("xT", [NT * P * 2, B], f8, kind="ExternalInput")
    w_d = nc.dram_tensor("w", [NQ * NT * P * 2, 512], f8,
                         kind="ExternalInput")
    o_d = nc.dram_tensor("out", [B, U], f16, kind="ExternalOutput")

    x_ap = x_d[:].rearrange("(c p k) m -> c p k m", p=P, k=2)  # [8,128,2,B]
    w_ap = w_d[:].rearrange("(c p k) u -> c p k u", p=P, k=4)  # [16,128,4,512]
    # same DRAM bytes viewed as 1MB oct chunks (two k-quads): used for
    # quarters 1-3, halving tile count and chain-boundary sem waits
    w_oap = w_d[:].rearrange("(q h t p k) u -> (q h) p t k u",
                             q=NQ, h=2, t=2, p=P, k=4)       # [8,128,2,4,512]
    o_ap = o_d[:].rearrange("(j p) u -> j p u", p=P)           # [MT, 128, U]

    DR = mybir.MatmulPerfMode.DoubleRow

    with tile.TileContext(nc) as tc, \
         tc.tile_pool(name="resident", bufs=1) as resident, \
         tc.tile_pool(name="mpsum", bufs=8, space="PSUM") as mpsum:

        xk = [resident.tile([P, 2, B], f8, name=f"xk_{t}")
              for t in range(1, NT)]
        xk.insert(0, None)
        # first x chunk split in half so the first matmul can start
        # ~1us earlier
        xk0h = [resident.tile([P, 2, B // 2], f8, name=f"xk0_{h}")
                for h in range(2)]
        # w quarter 0 arrives as k-quads (2KB/partition descriptors),
        # the first quad split into two pair tiles so the first
        # matmul can start early; quarters 1-3 arrive as 1MB octs
        # (same 2KB descriptors, fewer tiles).
        wq4 = [None if t == 0 else
               resident.tile([P, 4, 512], f8, name=f"wq4_0_{t}")
               for t in range(NT // 2)]
        wq0h = [resident.tile([P, 2, 512], f8, name=f"wq0_{h}")
                for h in range(2)]
        wq8o = [[resident.tile([P, 2, 4, 512], f8, name=f"wq8o_{q}_{h}")
                 for h in range(2)] for q in range(1, NQ)]
        ost = [resident.tile([P, U], f16, name=f"ost_{j}")
               for j in range(MT)]

        # ---- input DMAs: w on the sync ring, x on the scalar ring.
        # (Interleaving early x chunks onto the sync ring was tried
        # and regressed: the first ~5us of DMA bandwidth is a shared
        # ramp, so it only starved the scalar ring's x chunks.) ----
        # (A 1KB priming transfer at the head of each ring was tried
        # to absorb the ~1.5us issue-to-first-packet latency; the
        # latency is per-transfer descriptor pipelining, not ring
        # wake-up, so priming only delayed the real chunks.)
        nc.sync.dma_start(wq0h[0], w_ap[0][:, 0:2, :])
        nc.scalar.dma_start(xk0h[0], x_ap[0][:, :, :B // 2])
        nc.scalar.dma_start(xk0h[1], x_ap[0][:, :, B // 2:])
        nc.sync.dma_start(wq0h[1], w_ap[0][:, 2:4, :])
        for t in range(NT // 2):
            if t > 0:
                nc.sync.dma_start(wq4[t], w_ap[t])
                nc.scalar.dma_start(xk[2 * t], x_ap[2 * t])
            nc.scalar.dma_start(xk[2 * t + 1], x_ap[2 * t + 1])
        for q in range(1, NQ):
            for h in range(2):
                nc.sync.dma_start(wq8o[q - 1][h], w_oap[q * 2 + h])

        # ---- PE prewarm: dummy matmuls on memset data fill the PE
        # while the first input chunks are in flight and start the HAM
        # clock-gate busy window early. Sized to end right as the
        # first chunks land (dummies queue ahead of real matmuls, so
        # more is not better). ----
        if prewarm:
            # Memset-gated dummies start at ~8.5us, leaving the PE
            # cold (1.2GHz) until ~13.8us -- and that is intentional:
            # a PE warmed earlier (tried via memset-free dummies)
            # consumes chunks at 222GB/s, starves on the DMA ramp
            # (~50-150GB/s until ~14us), and the resulting stall
            # re-throttles the clock gate for another ~7us window.
            # The cold-PE chunk demand (~112GB/s) matches the ramp.
            dl = resident.tile([P, 2, P], f8, name="warm_l")
            dr_ = resident.tile([P, 2, 512], f8, name="warm_r")
            nc.gpsimd.memset(dl, 0)
            nc.gpsimd.memset(dr_, 0)
            wps = mpsum.tile([P, 512], f32, tag="ps", name="warm_ps")
            for i in range(prewarm):
                nc.tensor.matmul(wps, lhsT=dl, rhs=dr_,
                                 start=True, stop=True, perf_mode=DR)

        def mm(ps, j, q, t):
            if t == 0:
                lhsT = xk0h[j // 4][:, :, P * (j % 4):P * (j % 4 + 1)]
            else:
                lhsT = xk[t][:, :, P * j:P * (j + 1)]
            if q == 0:
                if t < 2:
                    rhs = wq0h[t]
                else:
                    kc = (t % 2) * 2
                    rhs = wq4[t // 2][:, kc:kc + 2, :]
            else:
                ti = t % 4
                kc = (ti % 2) * 2
                rhs = wq8o[q - 1][t // 4][:, ti // 2, kc:kc + 2, :]
            nc.tensor.matmul(
                ps, lhsT=lhsT, rhs=rhs,
                start=(t == 0), stop=(t == NT - 1),
                perf_mode=DR,
            )

        def evict_store(ps, j, q, store_engine=None, split=False):
            dst = ost[j][:, 512 * q:512 * (q + 1)]
            if split:
                # final chain: halve latency by evicting on both the
                # Vector and Scalar engines and storing the halves on
                # both DMA rings in parallel
                h = 256
                nc.vector.tensor_copy(dst[:, :h], ps[:, :h])
                nc.scalar.activation(
                    dst[:, h:], ps[:, h:],
                    mybir.ActivationFunctionType.Copy)
                nc.sync.dma_start(
                    o_ap[j, :, 512 * q:512 * q + h], dst[:, :h])
                nc.scalar.dma_start(
                    o_ap[j, :, 512 * q + h:512 * (q + 1)], dst[:, h:])
            else:
                nc.vector.tensor_copy(dst, ps)
                (store_engine or nc.scalar).dma_start(
                    o_ap[j, :, 512 * q:512 * (q + 1)], dst)

        # ---- phase 1: chains (j, q0), k-pair-outer, paced by arrival ----
        ps0 = [mpsum.tile([P, 512], f32, tag="ps", name=f"ps_{j}_0")
               for j in range(MT)]
        for t in range(NT):
            for j in range(MT):
                mm(ps0[j], j, 0, t)
        for j in range(MT):
            evict_store(ps0[j], j, 0)

        # ---- phase 2: chains (j, q1..3), q-major so each w quarter
        # is consumed long after it lands; stores alternate between
        # the two DMA rings (the sync ring is idle once w has landed)
        for q in range(1, NQ):
            for j in range(MT):
                if q == NQ - 1 and j == MT - 1:
                    continue
                ps = mpsum.tile([P, 512], f32, tag="ps",
                                name=f"ps_{j}_{q}")
                for t in range(NT):
                    mm(ps, j, q, t)
                evict_store(ps, j, q,
                            store_engine=nc.sync if j % 2 else nc.scalar)

        # The final chain runs as two independent half-width (N=256)
        # chains: the first half's evict + store overlap the second
        # half's matmuls, so only a 256-col evict and a 64KB store
        # remain after the last matmul.
        j, q = MT - 1, NQ - 1
        for h in range(2):
            ph = mpsum.tile([P, 512], f32, tag="ps", name=f"ps_f{h}")
            for t in range(NT):
                ti = t % 4
                kc = (ti % 2) * 2
                nc.tensor.matmul(
                    ph[:, :256],
                    lhsT=(xk0h[1][:, :, 3 * P:4 * P] if t == 0
                          else xk[t][:, :, P * j:P * (j + 1)]),
                    rhs=wq8o[q - 1][t // 4][:, ti // 2, kc:kc + 2,
                                            256 * h:256 * (h + 1)],
                    start=(t == 0), stop=(t == NT - 1),
                    perf_mode=DR,
                )
            dst = ost[j][:, 512 * q + 256 * h:512 * q + 256 * (h + 1)]
            if h == 0:
                nc.scalar.activation(dst, ph[:, :256],
                                     mybir.ActivationFunctionType.Copy)
                nc.sync.dma_start(
                    o_ap[j, :, 512 * q:512 * q + 256], dst)
            else:
                nc.vector.tensor_copy(dst, ph[:, :256])
                nc.scalar.dma_start(
                    o_ap[j, :, 512 * q + 256:512 * (q + 1)], dst)

    nc.compile()
    return nc


_NC_CACHE = {}
LAST_RESULTS = {}


def _get_nc(**kwargs):
    key = tuple(sorted(kwargs.items()))
    if key not in _NC_CACHE:
        _NC_CACHE[key] = build_kernel(**kwargs)
    return _NC_CACHE[key]


def kernel(x, w, _trace=False, _trace_cores=None, **build_kwargs):
    from concourse.bass_utils import run_bass_kernel_spmd
    import ml_dtypes

    x = np.asarray(x, dtype=np.float32)
    w = np.asarray(w, dtype=np.float32)
    assert x.shape == (B_FULL, D_IN) and w.shape == (D_IN, UNITS)

    nc = _get_nc(**build_kwargs)

    f8 = ml_dtypes.float8_e4m3
    # Exact host-side binarize: sign(v) with sign(0) -> +1, as +-1.0
    # which fp8e4m3 represents exactly.
    xb = np.where(x >= 0, np.float32(1), np.float32(-1))
    wb = np.where(w >= 0, np.float32(1), np.float32(-1))
    xbT8 = np.ascontiguousarray(xb.T).astype(f8)          # [D, B_FULL]
    # w chunk-major [q, t2, p, s4, u']: chunk (q,t2) holds k-rows
    # 512*t2 + 128*s4 + p of output-column quarter q, contiguous per
    # SBUF partition p (2KB descriptors).
    wq8 = np.ascontiguousarray(
        wb.astype(f8).reshape(4, 4, 128, 4, 512).transpose(3, 0, 2, 1, 4)
    ).reshape(4 * D_IN, 512)

    def x_chunked(xt):
        # [t, p, ko, m]: chunk t holds k-rows 256t + 128*ko + p
        return np.ascontiguousarray(
            xt.reshape(8, 2, 128, B_CORE).transpose(0, 2, 1, 3)
        ).reshape(D_IN, B_CORE)

    in_maps = [
        {"xT": x_chunked(xbT8[:, c * B_CORE:(c + 1) * B_CORE]),
         "w": wq8}
        for c in range(N_CORES)
    ]
    br = run_bass_kernel_spmd(
        nc, in_maps, list(range(N_CORES)),
        trace=_trace, trace_cores=_trace_cores,
    )
    LAST_RESULTS["br"] = br
    out = np.concatenate(
        [br.results[c]["out"].astype(np.float32) for c in range(N_CORES)],
        axis=0,
    )
    return out


if __name__ == "__main__":
    rng = np.random.default_rng(0)
    x = rng.standard_normal((B_FULL, D_IN), dtype=np.float32)
    w = (rng.standard_normal((D_IN, UNITS), dtype=np.float32) * 0.1).astype(
        np.float32
    )
    out = kernel(x, w)
    exp = np.sign(x + (x == 0)) @ np.sign(w + (w == 0))
    print("max abs err:", np.max(np.abs(out - exp)))
